# revision 23
# baseline (speedup 1.0000x reference)
"""Trainium2 Bass kernel for CdfgReader GNN message passing.

Strategy:
  - The GNN node features depend only on which CDFG a batch item references.
    With 64 batch items drawn from 32 CDFGs, compute the GNN once per UNIQUE
    graph (<=32) and distribute 4 graph slots per core across 8 cores.
  - Per graph slot: X0 = relu(xs @ W_in + b), 4 GCN layers
    (A @ (X @ W) + b with relu/tanh), residual, then per-batch masked mean
    via a small mask matmul. Each core emits the [64, 256] rows for the
    batch items whose graph it owns; the host gathers rows from owners.
  - Precision: X and W stay fp32 (fp32 matmuls for the small X@W work —
    rounding W to f32r alone costs 2.4e-2 end-to-end error). The dominant
    A-multiplies run in float32r (full PE rate): A is 0/1 (exact in f32r)
    and XW is split into hi+lo f32r parts on layers 0-2 so the product is
    fp32-accurate; layer 3 uses hi only. Measured end-to-end ~5e-5.
  - A^T is pre-transposed on the host (the PE contracts over the partition
    dim, and fp32 has no DMA-transpose path on TRN2).
"""

import os

import numpy as np

NG = 4          # graph slots per core
NCORES = 8
N = 1024        # max nodes
F = 128         # input feature dim
H = 256         # hidden dim
L = 4           # GCN layers
B = 64          # batch (coverpoints)

SPLIT_LAYERS = (0, 1, 2)   # A-mult layers using hi+lo split

_CACHE = {}


def _build_nc():
    import concourse.bass as bass  # noqa: F401
    import concourse.mybir as mybir
    import concourse.tile as tile
    from concourse import bacc
    from concourse.bass import ts

    f32 = mybir.dt.float32
    f32r = mybir.dt.float32r
    Relu = mybir.ActivationFunctionType.Relu
    Tanh = mybir.ActivationFunctionType.Tanh
    sub = mybir.AluOpType.subtract

    nc = bacc.Bacc("TRN2", target_bir_lowering=False, debug=False,
                   num_devices=NCORES)

    a_t = nc.dram_tensor("a_t", [NG, N, N], f32r, kind="ExternalInput")
    xs_t = nc.dram_tensor("xs_t", [F, NG, N], f32, kind="ExternalInput")
    m_t = nc.dram_tensor("m_t", [128, NG * 8, B], f32r, kind="ExternalInput")
    mask_full = nc.dram_tensor("mask_full", [B, N], f32, kind="ExternalInput")
    w_in = nc.dram_tensor("w_in", [F, H], f32, kind="ExternalInput")
    w_gcn_hi = nc.dram_tensor("w_gcn_hi", [128, L * 2, H], f32r,
                              kind="ExternalInput")
    w_gcn_lo = nc.dram_tensor("w_gcn_lo", [128, L * 2, H], f32r,
                              kind="ExternalInput")
    b_in_pp = nc.dram_tensor("b_in_pp", [128, 2], f32, kind="ExternalInput")
    b_gcn_pp = nc.dram_tensor("b_gcn_pp", [128, L * 2], f32, kind="ExternalInput")
    b_in_row = nc.dram_tensor("b_in_row", [1, H], f32r, kind="ExternalInput")
    b_g3_row = nc.dram_tensor("b_g3_row", [1, H], f32r, kind="ExternalInput")
    ones_row = nc.dram_tensor("ones_row", [1, 128], f32r, kind="ExternalInput")
    out = nc.dram_tensor("out", [B, H], f32, kind="ExternalOutput")

    with tile.TileContext(nc) as tc:
        with (
            tc.tile_pool(name="const", bufs=1) as constp,
            tc.tile_pool(name="apool", bufs=2) as apool,
            tc.tile_pool(name="xpool", bufs=2) as xpool,
            tc.tile_pool(name="xpool1", bufs=1) as xpool1,
            tc.tile_pool(name="psx", bufs=3, space="PSUM") as psx,
            tc.tile_pool(name="psw", bufs=3, space="PSUM") as psw,
            tc.tile_pool(name="psm", bufs=2, space="PSUM") as psm,
        ):
            # --- constants, loaded once ---
            w_in_sb = constp.tile([128, H], f32)
            nc.sync.dma_start(w_in_sb[:], w_in[:, :])
            w_hi_sb = constp.tile([128, L * 2, H], f32r)
            nc.sync.dma_start(w_hi_sb[:], w_gcn_hi[:, :, :])
            w_lo_sb = constp.tile([128, L * 2, H], f32r)
            nc.sync.dma_start(w_lo_sb[:], w_gcn_lo[:, :, :])
            b_in_pp_sb = constp.tile([128, 2], f32)
            nc.sync.dma_start(b_in_pp_sb[:], b_in_pp[:, :])
            b_gcn_pp_sb = constp.tile([128, L * 2], f32)
            nc.sync.dma_start(b_gcn_pp_sb[:], b_gcn_pp[:, :])
            b_in_row_sb = constp.tile([1, H], f32r)
            nc.sync.dma_start(b_in_row_sb[:], b_in_row[:, :])
            b_g3_row_sb = constp.tile([1, H], f32r)
            nc.sync.dma_start(b_g3_row_sb[:], b_g3_row[:, :])
            ones_sb = constp.tile([1, 128], f32r)
            nc.sync.dma_start(ones_sb[:], ones_row[:, :])
            m_t_sb = constp.tile([128, NG * 8, B], f32r)
            nc.sync.dma_start(m_t_sb[:], m_t[:, :, :])

            out_acc = constp.tile([B, H], f32)

            for g in range(NG):
                # A^T for this graph: 8 tiles [128(m), 1024(i)] in one tensor
                a_sb = apool.tile([128, 8, N], f32r, tag="a")
                nc.sync.dma_start(
                    a_sb[:], a_t[g].rearrange("(mo p) i -> p mo i", p=128))
                xs_g = xpool.tile([128, N], f32, tag="xs_g")
                nc.sync.dma_start(xs_g[:], xs_t[:, g, :])

                # X0^T hi/lo f32r companions (h-major) feed the split X@W
                # matmuls; the fp32 value only lives in a transient chunk.
                x0t_hi = xpool.tile([128, 2, N], f32r, tag="xh", name="x0t_hi")
                x0t_lo = xpool.tile([128, 2, N], f32r, tag="xl", name="x0t_lo")
                for t in range(2):
                    for c in range(2):
                        ps = psx.tile([128, 512], mybir.dt.float32, tag="psx")
                        nc.tensor.matmul(ps[:], w_in_sb[:, ts(t, 128)],
                                         xs_g[:, ts(c, 512)],
                                         start=True, stop=True)
                        xtmp = xpool.tile([128, 512], f32, tag="xtmp",
                                          name="x0tmp")
                        nc.scalar.activation(xtmp[:], ps[:],
                                             Relu, bias=b_in_pp_sb[:, t:t + 1])
                        nc.vector.tensor_copy(x0t_hi[:, t, ts(c, 512)],
                                              xtmp[:])
                        nc.vector.tensor_tensor(x0t_lo[:, t, ts(c, 512)],
                                                xtmp[:],
                                                x0t_hi[:, t, ts(c, 512)], sub)

                # X0 node-major fp32 (for the residual): [128, 8(i), 256(h)]
                x0n = xpool.tile([128, 8, H], f32, tag="x0n")
                for i in range(8):
                    ps = psw.tile([128, H], mybir.dt.float32, tag="psw")
                    nc.tensor.matmul(ps[:], xs_g[:, ts(i, 128)],
                                     w_in_sb[:], start=True, stop=False)
                    nc.tensor.matmul(ps[:], ones_sb[:], b_in_row_sb[:],
                                     start=False, stop=True)
                    nc.scalar.activation(x0n[:, i, :], ps[:], Relu)

                x_hi, x_lo = x0t_hi, x0t_lo
                xf = None
                for layer in range(L):
                    do_split = layer in SPLIT_LAYERS
                    # XW = X @ W_gcn[layer] via 3-way f32r split
                    # (X_hi@W_hi + X_lo@W_hi + X_hi@W_lo), then round/split
                    xw_hi = xpool.tile([128, 8, H], f32r, tag="xw_hi",
                                       name="xw_hi")
                    xw_lo = None
                    if do_split:
                        xw_lo = xpool1.tile([128, 8, H], f32r, tag="xw_lo",
                                            name="xw_lo")
                    for m in range(8):
                        ps = psw.tile([128, H], mybir.dt.float32, tag="psw")
                        k = 0
                        for t in range(2):
                            wh = w_hi_sb[:, layer * 2 + t, :]
                            wl = w_lo_sb[:, layer * 2 + t, :]
                            for lhsT, rhs in ((x_hi[:, t, ts(m, 128)], wh),
                                              (x_hi[:, t, ts(m, 128)], wl),
                                              (x_lo[:, t, ts(m, 128)], wh)):
                                nc.tensor.matmul(ps[:], lhsT, rhs,
                                                 start=(k == 0), stop=(k == 5))
                                k += 1
                        nc.vector.tensor_copy(xw_hi[:, m, :], ps[:])
                        if do_split:
                            nc.vector.tensor_tensor(
                                xw_lo[:, m, :], ps[:], xw_hi[:, m, :], sub)

                    parts = [xw_hi, xw_lo] if do_split else [xw_hi]
                    if layer < L - 1:
                        # X_next^T[h, i] = sum_m XW[m, h] * A^T[m, i]  (h-major)
                        xn_hi = xpool.tile([128, 2, N], f32r, tag="xh",
                                           name="xn_hi")
                        xn_lo = xpool.tile([128, 2, N], f32r, tag="xl",
                                           name="xn_lo")
                        for t in range(2):
                            pss = [psx.tile([128, 512], mybir.dt.float32,
                                            tag="psx", name=f"ps_{t}_{c}")
                                   for c in range(2)]
                            nmm = 8 * len(parts)
                            k = 0
                            for m in range(8):
                                for part in parts:
                                    for c in range(2):
                                        nc.tensor.matmul(
                                            pss[c][:], part[:, m, ts(t, 128)],
                                            a_sb[:, m, ts(c, 512)],
                                            start=(k == 0), stop=(k == nmm - 1))
                                    k += 1
                            for c in range(2):
                                xtmp = xpool.tile([128, 512], f32, tag="xtmp",
                                                  name="xtmp")
                                nc.scalar.activation(
                                    xtmp[:], pss[c][:], Relu,
                                    bias=b_gcn_pp_sb[:, layer * 2 + t:
                                                     layer * 2 + t + 1])
                                nc.vector.tensor_copy(
                                    xn_hi[:, t, ts(c, 512)], xtmp[:])
                                nc.vector.tensor_tensor(
                                    xn_lo[:, t, ts(c, 512)], xtmp[:],
                                    xn_hi[:, t, ts(c, 512)], sub)
                        x_hi, x_lo = xn_hi, xn_lo
                    else:
                        # Final layer node-major: X4[i, h] = sum_m A^T[m,i]^T XW[m,h]
                        xf = xpool1.tile([128, 8, H], f32r, tag="xf")
                        for i in range(8):
                            ps = psw.tile([128, H], mybir.dt.float32, tag="psw")
                            for m in range(8):
                                for part in parts:
                                    nc.tensor.matmul(
                                        ps[:], a_sb[:, m, ts(i, 128)],
                                        part[:, m, :],
                                        start=(m == 0 and part is parts[0]),
                                        stop=False)
                            nc.tensor.matmul(ps[:], ones_sb[:], b_g3_row_sb[:],
                                             start=False, stop=True)
                            nc.scalar.activation(ps[:], ps[:], Tanh)
                            # residual add; output rounds to f32r for mask mm
                            nc.vector.tensor_add(xf[:, i, :], ps[:],
                                                 x0n[:, i, :])

                # masked sums for the batch rows owned via this graph:
                # psum[b, h] += M^T[n, b]^T @ Xf[n, h]
                pm = psm.tile([B, H], mybir.dt.float32, tag="psm")
                for c in range(8):
                    nc.tensor.matmul(pm[:], m_t_sb[:, g * 8 + c, :],
                                     xf[:, c, :], start=(c == 0), stop=(c == 7))
                if g == 0:
                    nc.vector.tensor_copy(out_acc[:], pm[:])
                else:
                    nc.vector.tensor_add(out_acc[:], out_acc[:], pm[:])

            # --- epilogue: divide by per-batch mask count ---
            mask_sb = constp.tile([B, N], f32)
            nc.sync.dma_start(mask_sb[:], mask_full[:, :])
            cnt = constp.tile([B, 1], f32)
            nc.vector.reduce_sum(cnt[:], mask_sb[:], axis=mybir.AxisListType.X)
            inv = constp.tile([B, 1], f32)
            nc.vector.reciprocal(inv[:], cnt[:])
            out_sb = constp.tile([B, H], f32)
            nc.vector.tensor_scalar_mul(out_sb[:], out_acc[:], inv[:])
            nc.sync.dma_start(out[:, :], out_sb[:])

    nc.compile()
    return nc


def _get_nc():
    if "nc" not in _CACHE:
        _CACHE["nc"] = _build_nc()
    return _CACHE["nc"]


def _prepare_in_maps(cdfg_xs, cdfg_as, graph, coverpoint_mask,
                     W_in, b_in, W_gcn, b_gcn):
    cdfg_xs = np.asarray(cdfg_xs, dtype=np.float32)
    cdfg_as = np.asarray(cdfg_as, dtype=np.float32)
    graph = np.asarray(graph).astype(np.int64)
    maskf = np.asarray(coverpoint_mask).astype(np.float32)
    W_in = np.asarray(W_in, dtype=np.float32)
    b_in = np.asarray(b_in, dtype=np.float32)
    W_gcn = np.asarray(W_gcn, dtype=np.float32)
    b_gcn = np.asarray(b_gcn, dtype=np.float32)

    uniq = np.unique(graph)
    nslots = NG * NCORES
    slots = np.empty(nslots, dtype=np.int64)
    slots[:len(uniq)] = uniq
    slots[len(uniq):] = uniq[0]
    real = np.zeros(nslots, dtype=bool)
    real[:len(uniq)] = True

    def _rnd11(x):
        # round-to-nearest-even at 11 explicit mantissa bits (f32r-exact)
        m, e = np.frexp(np.float32(x))
        m = np.round(m * 4096.0) / 4096.0
        return np.ldexp(m, e).astype(np.float32)

    w_gcn_layout = np.ascontiguousarray(
        W_gcn.reshape(L, 2, 128, H).transpose(2, 0, 1, 3)
        .reshape(128, L * 2, H))
    w_gcn_hi = _rnd11(w_gcn_layout)
    w_gcn_lo = _rnd11(w_gcn_layout - w_gcn_hi)

    common = {
        "w_in": np.ascontiguousarray(W_in),                         # [128, 256]
        "w_gcn_hi": w_gcn_hi,
        "w_gcn_lo": w_gcn_lo,
        "b_in_pp": np.ascontiguousarray(b_in.reshape(2, 128).T),
        "b_gcn_pp": np.ascontiguousarray(
            b_gcn.reshape(L, 2, 128).transpose(2, 0, 1).reshape(128, L * 2)),
        "b_in_row": np.ascontiguousarray(b_in.reshape(1, H)),
        "b_g3_row": np.ascontiguousarray(b_gcn[L - 1].reshape(1, H)),
        "ones_row": np.ones((1, 128), dtype=np.float32),
        "mask_full": np.ascontiguousarray(maskf),
    }

    in_maps = []
    for k in range(NCORES):
        sl = slots[k * NG:(k + 1) * NG]
        a_t = np.empty((NG, N, N), dtype=np.float32)
        for g in range(NG):
            a_t[g] = cdfg_as[sl[g]].T
        xs_t = np.ascontiguousarray(cdfg_xs[sl].transpose(2, 0, 1))
        m_t = np.zeros((128, NG * 8, B), dtype=np.float32)
        for g in range(NG):
            if real[k * NG + g]:
                rows = np.nonzero(graph == sl[g])[0]
                for b in rows:
                    m_t[:, g * 8:(g + 1) * 8, b] = maskf[b].reshape(8, 128).T
        in_maps.append({"a_t": a_t, "xs_t": xs_t, "m_t": m_t, **common})
    return in_maps, slots, real


def _assemble_out(results, graph, slots, real):
    graph = np.asarray(graph).astype(np.int64)
    out = np.zeros((B, H), dtype=np.float32)
    for k in range(NCORES):
        for g in range(NG):
            if real[k * NG + g]:
                rows = graph == slots[k * NG + g]
                out[rows] = results[k]["out"][rows]
    return out


def kernel(cdfg_xs, cdfg_as, graph, coverpoint_mask, W_in, b_in, W_gcn, b_gcn):
    from concourse.bass_utils import run_bass_kernel_spmd

    nc = _get_nc()
    in_maps, slots, real = _prepare_in_maps(
        cdfg_xs, cdfg_as, graph, coverpoint_mask, W_in, b_in, W_gcn, b_gcn)
    res = run_bass_kernel_spmd(nc, in_maps, core_ids=list(range(NCORES)))
    return _assemble_out(res.results, graph, slots, real)


# revision 39
# speedup vs baseline: 7895.0788x; 7895.0788x over previous
"""Trainium2 Bass kernel for CdfgReader GNN message passing.

Strategy:
  - The GNN node features depend only on which CDFG a batch item references.
    With 64 batch items drawn from 32 CDFGs, compute the GNN once per UNIQUE
    graph (<=32) and distribute 4 graph slots per core across 8 cores.
  - Per graph slot: X0 = relu(xs @ W_in + b), 4 GCN layers
    (A @ (X @ W) + b with relu/tanh), residual, then per-batch masked mean
    via a small mask matmul. Each core emits the [64, 256] rows for the
    batch items whose graph it owns; the host gathers rows from owners.
  - Precision: X and W stay fp32 (fp32 matmuls for the small X@W work —
    rounding W to f32r alone costs 2.4e-2 end-to-end error). The dominant
    A-multiplies run in float32r (full PE rate): A is 0/1 (exact in f32r)
    and XW is split into hi+lo f32r parts on layers 0-2 so the product is
    fp32-accurate; layer 3 uses hi only. Measured end-to-end ~5e-5.
  - A^T is pre-transposed on the host (the PE contracts over the partition
    dim, and fp32 has no DMA-transpose path on TRN2).
"""

import os

import numpy as np

NG = 4          # graph slots per core
NCORES = 8
N = 1024        # max nodes
F = 128         # input feature dim
H = 256         # hidden dim
L = 4           # GCN layers
B = 64          # batch (coverpoints)

SPLIT_LAYERS = (0, 1, 2)   # A-mult layers using hi+lo split

_CACHE = {}


def _build_nc():
    import concourse.bass as bass  # noqa: F401
    import concourse.mybir as mybir
    import concourse.tile as tile
    from concourse import bacc
    from concourse.bass import ts

    f32 = mybir.dt.float32
    f32r = mybir.dt.float32r
    Relu = mybir.ActivationFunctionType.Relu
    Tanh = mybir.ActivationFunctionType.Tanh
    sub = mybir.AluOpType.subtract

    nc = bacc.Bacc("TRN2", target_bir_lowering=False, debug=False,
                   num_devices=NCORES)

    a_t = nc.dram_tensor("a_t", [NG, N, N], f32r, kind="ExternalInput")
    xs_t_hi = nc.dram_tensor("xs_t_hi", [F, NG, N], f32r, kind="ExternalInput")
    xs_t_lo = nc.dram_tensor("xs_t_lo", [F, NG, N], f32r, kind="ExternalInput")
    m_t = nc.dram_tensor("m_t", [128, NG * 8, B], f32r, kind="ExternalInput")
    mask_full = nc.dram_tensor("mask_full", [B, N], f32, kind="ExternalInput")
    w_in_hi = nc.dram_tensor("w_in_hi", [F, H], f32r, kind="ExternalInput")
    w_in_lo = nc.dram_tensor("w_in_lo", [F, H], f32r, kind="ExternalInput")
    w_gcn_hi = nc.dram_tensor("w_gcn_hi", [128, L * 2, H], f32r,
                              kind="ExternalInput")
    w_gcn_lo = nc.dram_tensor("w_gcn_lo", [128, L * 2, H], f32r,
                              kind="ExternalInput")
    b_in_pp = nc.dram_tensor("b_in_pp", [128, 2], f32, kind="ExternalInput")
    b_gcn_pp = nc.dram_tensor("b_gcn_pp", [128, L * 2], f32, kind="ExternalInput")
    b_in_row = nc.dram_tensor("b_in_row", [1, H], f32r, kind="ExternalInput")
    b_g3_row = nc.dram_tensor("b_g3_row", [1, H], f32r, kind="ExternalInput")
    ones_row = nc.dram_tensor("ones_row", [1, 128], f32r, kind="ExternalInput")
    out = nc.dram_tensor("out", [B, H], f32, kind="ExternalOutput")

    with tile.TileContext(nc) as tc:
        with (
            tc.tile_pool(name="const", bufs=1) as constp,
            tc.tile_pool(name="apool", bufs=2) as apool,
            tc.tile_pool(name="xpool", bufs=2) as xpool,
            tc.tile_pool(name="xpool1", bufs=1) as xpool1,
            tc.tile_pool(name="psx", bufs=4, space="PSUM") as psx,
            tc.tile_pool(name="psw", bufs=3, space="PSUM") as psw,
            tc.tile_pool(name="psm", bufs=1, space="PSUM") as psm,
        ):
            # --- constants, loaded once ---
            wi_hi_sb = constp.tile([128, H], f32r)
            nc.sync.dma_start(wi_hi_sb[:], w_in_hi[:, :])
            wi_lo_sb = constp.tile([128, H], f32r)
            nc.sync.dma_start(wi_lo_sb[:], w_in_lo[:, :])
            w_hi_sb = constp.tile([128, L * 2, H], f32r)
            nc.sync.dma_start(w_hi_sb[:], w_gcn_hi[:, :, :])
            w_lo_sb = constp.tile([128, L * 2, H], f32r)
            nc.sync.dma_start(w_lo_sb[:], w_gcn_lo[:, :, :])
            b_in_pp_sb = constp.tile([128, 2], f32)
            nc.sync.dma_start(b_in_pp_sb[:], b_in_pp[:, :])
            b_gcn_pp_sb = constp.tile([128, L * 2], f32)
            nc.sync.dma_start(b_gcn_pp_sb[:], b_gcn_pp[:, :])
            b_in_row_sb = constp.tile([1, H], f32r)
            nc.sync.dma_start(b_in_row_sb[:], b_in_row[:, :])
            b_g3_row_sb = constp.tile([1, H], f32r)
            nc.sync.dma_start(b_g3_row_sb[:], b_g3_row[:, :])
            ones_sb = constp.tile([1, 128], f32r)
            nc.sync.dma_start(ones_sb[:], ones_row[:, :])
            m_t_sb = constp.tile([128, NG * 8, B], f32r)
            nc.sync.dma_start(m_t_sb[:], m_t[:, :, :])

            out_acc = constp.tile([B, H], f32)

            for g in range(NG):
                # A^T for this graph: 8 tiles [128(m), 1024(i)] in one tensor
                a_sb = apool.tile([128, 8, N], f32r, tag="a")
                nc.sync.dma_start(
                    a_sb[:], a_t[g].rearrange("(mo p) i -> p mo i", p=128))
                xs_g_hi = xpool.tile([128, N], f32r, tag="xs_g_hi")
                nc.sync.dma_start(xs_g_hi[:], xs_t_hi[:, g, :])
                xs_g_lo = xpool.tile([128, N], f32r, tag="xs_g_lo")
                nc.sync.dma_start(xs_g_lo[:], xs_t_lo[:, g, :])

                # X0^T hi/lo f32r companions (h-major) feed the split X@W
                # matmuls; the fp32 value only lives in a transient chunk.
                x0t_hi = xpool.tile([128, 2, N], f32r, tag="xh", name="x0t_hi")
                x0t_lo = xpool.tile([128, 2, N], f32r, tag="xl", name="x0t_lo")
                for t in range(2):
                    for c in range(2):
                        ps = psx.tile([128, 512], mybir.dt.float32, tag="psx")
                        for k, (lhsT, rhs) in enumerate(
                                ((wi_hi_sb[:, ts(t, 128)], xs_g_hi[:, ts(c, 512)]),
                                 (wi_lo_sb[:, ts(t, 128)], xs_g_hi[:, ts(c, 512)]),
                                 (wi_hi_sb[:, ts(t, 128)], xs_g_lo[:, ts(c, 512)]))):
                            nc.tensor.matmul(ps[:], lhsT, rhs,
                                             start=(k == 0), stop=(k == 2))
                        xtmp = xpool.tile([128, 512], f32, tag="xtmp",
                                          name="x0tmp")
                        nc.scalar.activation(xtmp[:], ps[:],
                                             Relu, bias=b_in_pp_sb[:, t:t + 1])
                        nc.vector.tensor_copy(x0t_hi[:, t, ts(c, 512)],
                                              xtmp[:])
                        nc.vector.tensor_tensor(x0t_lo[:, t, ts(c, 512)],
                                                xtmp[:],
                                                x0t_hi[:, t, ts(c, 512)], sub)

                # X0 node-major fp32 (for the residual): [128, 8(i), 256(h)]
                x0n = xpool.tile([128, 8, H], f32, tag="x0n")
                for i in range(8):
                    ps = psw.tile([128, H], mybir.dt.float32, tag="psw")
                    for k, (lhsT, rhs) in enumerate(
                            ((xs_g_hi[:, ts(i, 128)], wi_hi_sb[:]),
                             (xs_g_hi[:, ts(i, 128)], wi_lo_sb[:]),
                             (xs_g_lo[:, ts(i, 128)], wi_hi_sb[:]))):
                        nc.tensor.matmul(ps[:], lhsT, rhs,
                                         start=(k == 0), stop=False)
                    nc.tensor.matmul(ps[:], ones_sb[:], b_in_row_sb[:],
                                     start=False, stop=True)
                    nc.scalar.activation(x0n[:, i, :], ps[:], Relu)

                x_hi, x_lo = x0t_hi, x0t_lo
                xf = None
                for layer in range(L):
                    do_split = layer in SPLIT_LAYERS
                    # XW = X @ W_gcn[layer] via 3-way f32r split
                    # (X_hi@W_hi + X_lo@W_hi + X_hi@W_lo), then round/split
                    xw_hi = xpool.tile([128, 8, H], f32r, tag="xw_hi",
                                       name="xw_hi")
                    xw_lo = None
                    if do_split:
                        xw_lo = xpool1.tile([128, 8, H], f32r, tag="xw_lo",
                                            name="xw_lo")
                    for m in range(8):
                        ps = psw.tile([128, H], mybir.dt.float32, tag="psw")
                        k = 0
                        for t in range(2):
                            wh = w_hi_sb[:, layer * 2 + t, :]
                            wl = w_lo_sb[:, layer * 2 + t, :]
                            for lhsT, rhs in ((x_hi[:, t, ts(m, 128)], wh),
                                              (x_hi[:, t, ts(m, 128)], wl),
                                              (x_lo[:, t, ts(m, 128)], wh)):
                                nc.tensor.matmul(ps[:], lhsT, rhs,
                                                 start=(k == 0), stop=(k == 5))
                                k += 1
                        nc.vector.tensor_copy(xw_hi[:, m, :], ps[:])
                        if do_split:
                            nc.vector.tensor_tensor(
                                xw_lo[:, m, :], ps[:], xw_hi[:, m, :], sub)

                    parts = [xw_hi, xw_lo] if do_split else [xw_hi]
                    if layer < L - 1:
                        # X_next^T[h, i] = sum_m XW[m, h] * A^T[m, i]  (h-major)
                        xn_hi = xpool.tile([128, 2, N], f32r, tag="xh",
                                           name="xn_hi")
                        xn_lo = xpool.tile([128, 2, N], f32r, tag="xl",
                                           name="xn_lo")
                        for t in range(2):
                            pss = [psx.tile([128, 512], mybir.dt.float32,
                                            tag="psx", name=f"ps_{t}_{c}")
                                   for c in range(2)]
                            nmm = 8 * len(parts)
                            k = 0
                            for m in range(8):
                                for part in parts:
                                    for c in range(2):
                                        nc.tensor.matmul(
                                            pss[c][:], part[:, m, ts(t, 128)],
                                            a_sb[:, m, ts(c, 512)],
                                            start=(k == 0), stop=(k == nmm - 1))
                                    k += 1
                            for c in range(2):
                                xtmp = xpool.tile([128, 512], f32, tag="xtmp",
                                                  name="xtmp")
                                nc.scalar.activation(
                                    xtmp[:], pss[c][:], Relu,
                                    bias=b_gcn_pp_sb[:, layer * 2 + t:
                                                     layer * 2 + t + 1])
                                nc.vector.tensor_copy(
                                    xn_hi[:, t, ts(c, 512)], xtmp[:])
                                nc.vector.tensor_tensor(
                                    xn_lo[:, t, ts(c, 512)], xtmp[:],
                                    xn_hi[:, t, ts(c, 512)], sub)
                        x_hi, x_lo = xn_hi, xn_lo
                    else:
                        # Final layer node-major: X4[i, h] = sum_m A^T[m,i]^T XW[m,h]
                        xf = xpool1.tile([128, 8, H], f32r, tag="xf")
                        for i in range(8):
                            ps = psw.tile([128, H], mybir.dt.float32, tag="psw")
                            for m in range(8):
                                for part in parts:
                                    nc.tensor.matmul(
                                        ps[:], a_sb[:, m, ts(i, 128)],
                                        part[:, m, :],
                                        start=(m == 0 and part is parts[0]),
                                        stop=False)
                            nc.tensor.matmul(ps[:], ones_sb[:], b_g3_row_sb[:],
                                             start=False, stop=True)
                            nc.scalar.activation(ps[:], ps[:], Tanh)
                            # residual add; output rounds to f32r for mask mm
                            nc.vector.tensor_add(xf[:, i, :], ps[:],
                                                 x0n[:, i, :])

                # masked sums for the batch rows owned via this graph:
                # psum[b, h] += M^T[n, b]^T @ Xf[n, h]
                pm = psm.tile([B, H], mybir.dt.float32, tag="psm")
                for c in range(8):
                    nc.tensor.matmul(pm[:], m_t_sb[:, g * 8 + c, :],
                                     xf[:, c, :], start=(c == 0), stop=(c == 7))
                if g == 0:
                    nc.vector.tensor_copy(out_acc[:], pm[:])
                else:
                    nc.vector.tensor_add(out_acc[:], out_acc[:], pm[:])

            # --- epilogue: divide by per-batch mask count ---
            mask_sb = constp.tile([B, N], f32)
            nc.sync.dma_start(mask_sb[:], mask_full[:, :])
            cnt = constp.tile([B, 1], f32)
            nc.vector.reduce_sum(cnt[:], mask_sb[:], axis=mybir.AxisListType.X)
            inv = constp.tile([B, 1], f32)
            nc.vector.reciprocal(inv[:], cnt[:])
            out_sb = constp.tile([B, H], f32)
            nc.vector.tensor_scalar_mul(out_sb[:], out_acc[:], inv[:])
            nc.sync.dma_start(out[:, :], out_sb[:])

    nc.compile()
    return nc


def _get_nc():
    if "nc" not in _CACHE:
        _CACHE["nc"] = _build_nc()
    return _CACHE["nc"]


def _prepare_in_maps(cdfg_xs, cdfg_as, graph, coverpoint_mask,
                     W_in, b_in, W_gcn, b_gcn):
    cdfg_xs = np.asarray(cdfg_xs, dtype=np.float32)
    cdfg_as = np.asarray(cdfg_as, dtype=np.float32)
    graph = np.asarray(graph).astype(np.int64)
    maskf = np.asarray(coverpoint_mask).astype(np.float32)
    W_in = np.asarray(W_in, dtype=np.float32)
    b_in = np.asarray(b_in, dtype=np.float32)
    W_gcn = np.asarray(W_gcn, dtype=np.float32)
    b_gcn = np.asarray(b_gcn, dtype=np.float32)

    uniq = np.unique(graph)
    nslots = NG * NCORES
    slots = np.empty(nslots, dtype=np.int64)
    slots[:len(uniq)] = uniq
    slots[len(uniq):] = uniq[0]
    real = np.zeros(nslots, dtype=bool)
    real[:len(uniq)] = True

    def _rnd11(x):
        # round-to-nearest-even at 11 explicit mantissa bits (f32r-exact)
        m, e = np.frexp(np.float32(x))
        m = np.round(m * 4096.0) / 4096.0
        return np.ldexp(m, e).astype(np.float32)

    w_gcn_layout = np.ascontiguousarray(
        W_gcn.reshape(L, 2, 128, H).transpose(2, 0, 1, 3)
        .reshape(128, L * 2, H))
    w_gcn_hi = _rnd11(w_gcn_layout)
    w_gcn_lo = _rnd11(w_gcn_layout - w_gcn_hi)
    w_in_hi = _rnd11(W_in)
    w_in_lo = _rnd11(W_in - w_in_hi)

    common = {
        "w_in_hi": np.ascontiguousarray(w_in_hi),
        "w_in_lo": np.ascontiguousarray(w_in_lo),
        "w_gcn_hi": w_gcn_hi,
        "w_gcn_lo": w_gcn_lo,
        "b_in_pp": np.ascontiguousarray(b_in.reshape(2, 128).T),
        "b_gcn_pp": np.ascontiguousarray(
            b_gcn.reshape(L, 2, 128).transpose(2, 0, 1).reshape(128, L * 2)),
        "b_in_row": np.ascontiguousarray(b_in.reshape(1, H)),
        "b_g3_row": np.ascontiguousarray(b_gcn[L - 1].reshape(1, H)),
        "ones_row": np.ones((1, 128), dtype=np.float32),
        "mask_full": np.ascontiguousarray(maskf),
    }

    in_maps = []
    for k in range(NCORES):
        sl = slots[k * NG:(k + 1) * NG]
        a_t = np.empty((NG, N, N), dtype=np.float32)
        for g in range(NG):
            a_t[g] = cdfg_as[sl[g]].T
        xs_t = np.ascontiguousarray(cdfg_xs[sl].transpose(2, 0, 1))
        xs_t_hi = _rnd11(xs_t)
        xs_t_lo = _rnd11(xs_t - xs_t_hi)
        m_t = np.zeros((128, NG * 8, B), dtype=np.float32)
        for g in range(NG):
            if real[k * NG + g]:
                rows = np.nonzero(graph == sl[g])[0]
                for b in rows:
                    m_t[:, g * 8:(g + 1) * 8, b] = maskf[b].reshape(8, 128).T
        in_maps.append({"a_t": a_t, "xs_t_hi": xs_t_hi, "xs_t_lo": xs_t_lo,
                        "m_t": m_t, **common})
    return in_maps, slots, real


def _assemble_out(results, graph, slots, real):
    graph = np.asarray(graph).astype(np.int64)
    out = np.zeros((B, H), dtype=np.float32)
    for k in range(NCORES):
        for g in range(NG):
            if real[k * NG + g]:
                rows = graph == slots[k * NG + g]
                out[rows] = results[k]["out"][rows]
    return out


def kernel(cdfg_xs, cdfg_as, graph, coverpoint_mask, W_in, b_in, W_gcn, b_gcn):
    from concourse.bass_utils import run_bass_kernel_spmd

    nc = _get_nc()
    in_maps, slots, real = _prepare_in_maps(
        cdfg_xs, cdfg_as, graph, coverpoint_mask, W_in, b_in, W_gcn, b_gcn)
    res = run_bass_kernel_spmd(nc, in_maps, core_ids=list(range(NCORES)))
    return _assemble_out(res.results, graph, slots, real)
